# Initial kernel scaffold
#
"""LIF neuron (leaky integrate-and-fire) Bass kernel for Trainium2.

Reference semantics (per element, recurrence over time axis T=32):
    mem_t   = tau * mem_{t-1} + x_t
    spike_t = 1.0 if mem_t > vth else 0.0
    mem_t   = mem_t * (1 - spike_t)        # hard reset

Input  x: [16, 32, 65536] f32  ->  Output spikes: [16, 32, 65536] f32.

Sharding: data parallel over batch, 8 cores x 2 batch rows each.

Schedule (per pass): ALL DMA rides the sync HWDGE ring. Mixed-direction
HBM traffic costs ~13% (measured: loads alone 43.8us @383GB/s, stores
alone ~50us @335GB/s, concurrent 106.7us), so the FIFO ring is loaded
with [all 16 MiB of loads][all 16 MiB of stores]: HBM stays
direction-pure, and in the hw_loop the next pass's loads queue behind
this pass's stores. Spikes are written in-place over the consumed x
tile so the whole input stays SBUF-resident (8 groups x 2 MiB = 16 MiB
+ 4 MiB work pools < 26 MiB usable).

DRAM layouts (independent knobs, host-side permute in kernel()):
  btd: [B_SH, T, D] natural layout; per-(group, batch) 1 MiB DMAs with
       2 KiB per-partition runs inside a contiguous 1 MiB window.
  gp:  [NG, P, JG] group-major; per-group single 2 MiB DMA with 16 KiB
       per-partition runs inside a contiguous 2 MiB window.
(Measured: fully partition-major [P, J] hurts loads 43.8->51.0us --
scattered 128 KiB-strided chunks lose HBM locality; stores are
insensitive, ~50us in every layout.)

Per step ([128, 1024] f32 tile, 2 batch x 512 d-elems per partition):
  DVE  scalar_tensor_tensor: acc = (mem * tau) + x_t
  ACT  Sign:  sgn = sign(acc - vth)
  ACT  Relu:  spk = relu(sgn)          (in-place over x_t)
  DVE  scalar_tensor_tensor: mem' = (acc <= vth) * acc
"""

import os
import sys

sys.path.insert(0, "/opt/trn_rl_repo")

import numpy as np

from concourse import bacc, mybir, tile
from concourse.bass_utils import run_bass_kernel_spmd

TAU = 0.2
VTH = 0.5

B, T, D = 16, 32, 65536
N_CORES = 8
B_SH = B // N_CORES          # 2 batch rows per core
P = 128                      # SBUF partitions
FB = D // P                  # 512 d-elems per partition per batch row
F = B_SH * FB                # 1024 free elems per step-tile
J = T * F                    # 32768 per-partition elements per core

GS = int(os.environ.get("LIF_GS", "4"))   # timesteps per DMA group
NG = T // GS                 # groups per pass
JG = GS * F                  # per-group free elems (4096)

IN_LAYOUT = os.environ.get("LIF_INL", "btd")    # btd | gp
OUT_LAYOUT = os.environ.get("LIF_OUTL", "btd")  # btd | gp
DUAL_LOADS = os.environ.get("LIF_DL", "0") == "1"   # odd-group loads on scalar ring
DUAL_STORES = os.environ.get("LIF_DS", "0") == "1"  # odd-group stores on gpsimd (SWDGE)
# First TRICKLE groups' stores ride the scalar ring inline with compute,
# draining during the (write-idle) load phase instead of extending the
# store phase. Measured dose-response: K=0 {98.8, 101.3, 102.5}us,
# K=2 {98.7, 105.5}us (indistinguishable from K=0 within machine noise),
# K=3 111us (mixing penalty dominates). Default 0: phase-pure, most
# validated.
TRICKLE = int(os.environ.get("LIF_TK", "0"))
# Store-split: emit each (group, batch) store as SS chunks along tl.
# SS=2 -> 512 KiB store DMAs (best measured pure-store rate: 50.0us vs
# 51.2us at 1 MiB).
SS = int(os.environ.get("LIF_SS", "1"))

_progs = {}


def _dram_decl(nc, name, layout, kind):
    f32 = mybir.dt.float32
    if layout == "btd":
        t = nc.dram_tensor(name, [B_SH, T, D], f32, kind=kind).ap()
        r = t.rearrange("b (g tl) (p f) -> g p tl b f", tl=GS, p=P)
        # per-group list of (dram AP, sbuf selector) DMA pairs
        def pairs(g, xt_v):
            return [(r[g][:, :, b], xt_v[:, :, b]) for b in range(B_SH)]
    elif layout == "gb":
        # (group, batch)-windows laid out consecutively: the per-DMA
        # pattern is identical to btd (1 MiB window, [p, tl, f]), but
        # successive DMAs sweep DRAM monotonically with zero jumps
        t = nc.dram_tensor(name, [NG * B_SH * GS, D], f32, kind=kind).ap()
        r = t.rearrange("(g b tl) (p f) -> g b p tl f", b=B_SH, tl=GS, p=P)
        def pairs(g, xt_v):
            return [(r[g][b], xt_v[:, :, b]) for b in range(B_SH)]
    else:  # gp
        t = nc.dram_tensor(name, [NG, P, JG], f32, kind=kind).ap()
        r = t.rearrange("g p j -> g p j")
        def pairs(g, xt_v):
            return [(r[g], None)]  # None -> use whole tile
    return pairs


def _build_program(hw_loop=None, mode="full"):
    f32 = mybir.dt.float32
    nc = bacc.Bacc(
        "TRN2",
        target_bir_lowering=False,
        debug=False,
        enable_asserts=False,
        num_devices=N_CORES,
    )
    in_pairs = _dram_decl(nc, "x", IN_LAYOUT, "ExternalInput")
    out_pairs = _dram_decl(nc, "out", OUT_LAYOUT, "ExternalOutput")

    with tile.TileContext(nc) as tc:
        with (
            # full-x residency: one buffer per group, reused across passes
            tc.tile_pool(name="xt", bufs=NG) as xp,
            tc.tile_pool(name="acc", bufs=3) as ap_,
            tc.tile_pool(name="sgn", bufs=3) as gp_,
            tc.tile_pool(name="mem", bufs=2) as mp,
            tc.tile_pool(name="const", bufs=1) as cp,
        ):
            nvth = cp.tile([P, 1], f32)
            nc.gpsimd.memset(nvth[:], -VTH)

            src = None
            if mode in ("store", "store2", "store3", "storesync", "dmanodep"):
                # constant source tile for dependency-free store microbench
                src = cp.tile([P, JG], f32)
                nc.gpsimd.memset(src[:], 0.125)

            def body():
                if mode == "full":
                    one_pass_phase(
                        nc, tc, in_pairs, out_pairs, xp, ap_, gp_, mp, nvth
                    )
                elif mode == "compute":
                    compute_pass(nc, xp, ap_, gp_, mp, nvth)
                else:
                    micro_pass(nc, in_pairs, out_pairs, xp, src, mode)

            if hw_loop is None:
                body()
            else:
                # benchmarking only: repeat the full pass in a HW loop so
                # per-pass device time can be fit from wall-clock deltas
                with tc.For_i(0, hw_loop, 1):
                    body()
    nc.compile()
    return nc


def _load_group(nc, eng, in_pairs, g, xt, xt_v):
    for dram, sel in in_pairs(g, xt_v):
        eng.dma_start(out=(xt[:] if sel is None else sel), in_=dram)


def _store_group(nc, eng, out_pairs, g, src_full, src_v):
    for dram, sel in out_pairs(g, src_v):
        eng.dma_start(out=dram, in_=(src_full if sel is None else sel))


def _lif_steps(nc, xt, g, ap_, gp_, mp, nvth, mem):
    """Emit the GS recurrence steps for group g; spikes in-place in xt."""
    f32 = mybir.dt.float32
    mult = mybir.AluOpType.mult
    add = mybir.AluOpType.add
    is_le = mybir.AluOpType.is_le
    Sign = mybir.ActivationFunctionType.Sign
    Relu = mybir.ActivationFunctionType.Relu
    for tl in range(GS):
        t = g * GS + tl
        xs = xt[:, tl * F : (tl + 1) * F]
        if t == 0:
            acc = xs  # mem_{-1} = 0 -> acc = x_0
        else:
            acc = ap_.tile([P, F], f32)
            # acc = (mem * tau) + x_t
            nc.vector.scalar_tensor_tensor(
                out=acc[:], in0=mem[:], scalar=TAU, in1=xs,
                op0=mult, op1=add,
            )
        sgn = gp_.tile([P, F], f32)
        # sgn = sign(acc-vth); relu(sgn) = (acc > vth) exactly
        nc.scalar.activation(sgn[:], acc[:], Sign, bias=nvth[:])
        if t < T - 1:
            mem = mp.tile([P, F], f32)
            # mem' = (acc <= vth) * acc   (hard reset)
            nc.vector.scalar_tensor_tensor(
                out=mem[:], in0=acc[:], scalar=VTH, in1=acc[:],
                op0=is_le, op1=mult,
            )
        # spike_t overwrites x_t (in-place; x_t is dead after acc/mem)
        nc.scalar.activation(xt[:, tl * F : (tl + 1) * F], sgn[:], Relu)
    return mem


def one_pass_phase(nc, tc, in_pairs, out_pairs, xp, ap_, gp_, mp, nvth):
    """Phase-pure schedule. Base: everything on the sync ring, FIFO
    gives [loads][stores]. Optional second rings need explicit gates
    (chain_iter_dep) to preserve direction purity:
      DUAL_LOADS: odd-group loads ride the scalar ring. Gates: first
        scalar load <- prev pass's last store ("sl"); first store <-
        last scalar load ("ls"). The scalar engine dispatches its load
        instructions before its compute stream; their WAR sems are met
        during the previous store phase, so ACT compute is not stalled.
      DUAL_STORES: odd-group stores ride the gpsimd SWDGE queue, all
        emitted at the end of the body (Pool is otherwise idle). Gates:
        first gpsimd store <- last load ("lg"); next pass's sync loads
        follow by ring FIFO, scalar loads by "sg".
    """
    f32 = mybir.dt.float32
    # Phase 1: issue every load
    xts = []
    first_sc_load = last_sc_load = None
    last_load = None
    for g in range(NG):
        xt = xp.tile([P, JG], f32)
        xt_v = xt[:].rearrange("p (tl b f) -> p tl b f", tl=GS, b=B_SH)
        eng = nc.scalar if (DUAL_LOADS and g % 2) else nc.sync
        ops = []
        for dram, sel in in_pairs(g, xt_v):
            op = eng.dma_start(out=(xt[:] if sel is None else sel), in_=dram)
            ops.append(op)
        if DUAL_LOADS and g % 2:
            if first_sc_load is None:
                first_sc_load = ops[0]
                tc.chain_iter_dep("sl", ops[0].ins)
                if DUAL_STORES:
                    tc.chain_iter_dep("sg", ops[0].ins)
            last_sc_load = ops[-1]
        last_load = ops[-1]
        xts.append((xt, xt_v))
    gate_load = last_sc_load if DUAL_LOADS else last_load
    if DUAL_LOADS:
        tc.chain_iter_dep("ls", gate_load.ins)
    if DUAL_STORES:
        tc.chain_iter_dep("lg", gate_load.ins)

    # Phase 2: recurrence; sync-ring stores drain after loads by FIFO
    mem = None
    first_st = last_st = None
    for g in range(NG):
        xt, xt_v = xts[g]
        mem = _lif_steps(nc, xt, g, ap_, gp_, mp, nvth, mem)
        if DUAL_STORES and g % 2:
            continue  # stored below on the gpsimd queue
        st_eng = nc.scalar if g < TRICKLE else nc.sync
        for dram, sel in out_pairs(g, xt_v):
            if SS > 1 and sel is not None:
                step = GS // SS
                chunks = [
                    (dram[:, c * step : (c + 1) * step],
                     sel[:, c * step : (c + 1) * step])
                    for c in range(SS)
                ]
            else:
                chunks = [(dram, xt[:] if sel is None else sel)]
            for dchunk, schunk in chunks:
                op = st_eng.dma_start(out=dchunk, in_=schunk)
                if g >= TRICKLE:
                    if first_st is None:
                        first_st = op
                        if DUAL_LOADS:
                            tc.chain_iter_dep("ls", op.ins)
                    last_st = op
    if DUAL_STORES:
        first_gp = None
        for g in range(1, NG, 2):
            xt, xt_v = xts[g]
            for dram, sel in out_pairs(g, xt_v):
                op = nc.gpsimd.dma_start(
                    out=dram, in_=(xt[:] if sel is None else sel)
                )
                if first_gp is None:
                    first_gp = op
                    tc.chain_iter_dep("lg", op.ins)
                last_gp = op
        if DUAL_LOADS:
            tc.chain_iter_dep("sg", last_gp.ins)
    if DUAL_LOADS:
        tc.chain_iter_dep("sl", last_st.ins)


def compute_pass(nc, xp, ap_, gp_, mp, nvth):
    """Compute-only microbench: fabricate x via memset, no DMA."""
    f32 = mybir.dt.float32
    mem = None
    for g in range(NG):
        xt = xp.tile([P, JG], f32)
        nc.gpsimd.memset(xt[:], 0.125)
        mem = _lif_steps(nc, xt, g, ap_, gp_, mp, nvth, mem)


def micro_pass(nc, in_pairs, out_pairs, xp, src, mode):
    """DMA-throughput microbenches.

    load:      input loads only (sync ring)
    store:     stores only from a constant tile (scalar ring)
    storesync: stores only, sync ring
    phasedma:  [all loads][all stores] on sync ring, stores read the
               loaded tiles -- pure-DMA floor of the phase schedule
    dmanodep:  loads on sync + stores on scalar, no cross deps
    """
    f32 = mybir.dt.float32
    if mode == "load32":
        # two back-to-back 16 MiB load sweeps, no stores: separates
        # per-phase ramp cost from direction-flip cost
        for _rep in range(2):
            for g in range(NG):
                xt = xp.tile([P, JG], f32)
                xt_v = xt[:].rearrange(
                    "p (tl b f) -> p tl b f", tl=GS, b=B_SH
                )
                _load_group(nc, nc.sync, in_pairs, g, xt, xt_v)
        return
    if mode == "phasedma":
        xts = []
        for g in range(NG):
            xt = xp.tile([P, JG], f32)
            xt_v = xt[:].rearrange(
                "p (tl b f) -> p tl b f", tl=GS, b=B_SH
            )
            _load_group(nc, nc.sync, in_pairs, g, xt, xt_v)
            xts.append((xt, xt_v))
        for g in range(NG):
            xt, xt_v = xts[g]
            _store_group(nc, nc.sync, out_pairs, g, xt[:], xt_v)
        return
    do_load = mode in ("load", "load2", "dmanodep")
    do_store = mode in ("store", "store2", "store3", "storesync", "dmanodep")
    src_v = None
    if src is not None:
        src_v = src[:].rearrange("p (tl b f) -> p tl b f", tl=GS, b=B_SH)
    for g in range(NG):
        if do_load:
            xt = xp.tile([P, JG], f32)
            xt_v = xt[:].rearrange(
                "p (tl b f) -> p tl b f", tl=GS, b=B_SH
            )
            eng = nc.scalar if (mode == "load2" and g % 2) else nc.sync
            _load_group(nc, eng, in_pairs, g, xt, xt_v)
        if do_store:
            if mode == "storesync":
                eng = nc.sync
            elif mode == "store2":
                eng = nc.sync if g % 2 else nc.scalar
            elif mode == "store3":
                eng = nc.gpsimd if g % 2 else nc.sync
            else:
                eng = nc.scalar
            _store_group(nc, eng, out_pairs, g, src[:], src_v)


def _get_program(hw_loop=None, mode="full"):
    key = (hw_loop, mode)
    if key not in _progs:
        _progs[key] = _build_program(hw_loop, mode)
    return _progs[key]


# ---- host-side layout transforms -------------------------------------

def _to_gp(xc):
    """[B_SH, T, D] -> [NG, P, JG] group-major partition-contiguous."""
    # x3[g, p, tl*F + b*FB + f] = x[b, g*GS+tl, p*FB+f]
    return np.ascontiguousarray(
        xc.reshape(B_SH, NG, GS, P, FB)
        .transpose(1, 3, 2, 0, 4)
        .reshape(NG, P, JG)
    )


def _from_gp(oc):
    """[NG, P, JG] -> [B_SH, T, D]: inverse of _to_gp."""
    return np.ascontiguousarray(
        oc.reshape(NG, P, GS, B_SH, FB)
        .transpose(3, 0, 2, 1, 4)
        .reshape(B_SH, T, D)
    )


def _to_gb(xc):
    """[B_SH, T, D] -> [NG*B_SH*GS, D]: consecutive (g, b) windows."""
    return np.ascontiguousarray(
        xc.reshape(B_SH, NG, GS, D)
        .transpose(1, 0, 2, 3)
        .reshape(NG * B_SH * GS, D)
    )


def _from_gb(oc):
    """Inverse of _to_gb."""
    return np.ascontiguousarray(
        oc.reshape(NG, B_SH, GS, D)
        .transpose(1, 0, 2, 3)
        .reshape(B_SH, T, D)
    )


def _shard_input(xc):
    if IN_LAYOUT == "btd":
        return np.ascontiguousarray(xc)
    if IN_LAYOUT == "gb":
        return _to_gb(xc)
    return _to_gp(xc)


def _gather_output(oc):
    if OUT_LAYOUT == "btd":
        return oc
    if OUT_LAYOUT == "gb":
        return _from_gb(oc)
    return _from_gp(oc)


def device_input(x):
    """Full [B, T, D] -> axis-0 shard-concatenated device input array."""
    return np.concatenate(
        [
            _shard_input(x[i * B_SH : (i + 1) * B_SH])
            for i in range(N_CORES)
        ],
        axis=0,
    )


def device_output(o):
    """Axis-0 shard-concatenated device output -> full [B, T, D]."""
    rows = o.shape[0] // N_CORES
    return np.concatenate(
        [
            _gather_output(o[i * rows : (i + 1) * rows])
            for i in range(N_CORES)
        ],
        axis=0,
    )


def _shard(x):
    return [
        {"x": _shard_input(x[i * B_SH : (i + 1) * B_SH])}
        for i in range(N_CORES)
    ]


def kernel(x):
    x = np.asarray(x, dtype=np.float32)
    assert x.shape == (B, T, D), x.shape
    nc = _get_program()
    res = run_bass_kernel_spmd(nc, _shard(x), list(range(N_CORES)))
    return np.concatenate(
        [_gather_output(res.results[i]["out"]) for i in range(N_CORES)],
        axis=0,
    )



# revision 1
# speedup vs baseline: 1.1294x; 1.1294x over previous
"""LIF neuron (leaky integrate-and-fire) Bass kernel for Trainium2.

Reference semantics (per element, recurrence over time axis T=32):
    mem_t   = tau * mem_{t-1} + x_t
    spike_t = 1.0 if mem_t > vth else 0.0
    mem_t   = mem_t * (1 - spike_t)        # hard reset

Input  x: [16, 32, 65536] f32  ->  Output spikes: [16, 32, 65536] f32.

Sharding: data parallel over batch, 8 cores x 2 batch rows each.

Schedule (per pass): ALL DMA rides the sync HWDGE ring. Mixed-direction
HBM traffic costs ~13% (measured: loads alone 43.8us @383GB/s, stores
alone ~50us @335GB/s, concurrent 106.7us), so the FIFO ring is loaded
with [all 16 MiB of loads][all 16 MiB of stores]: HBM stays
direction-pure, and in the hw_loop the next pass's loads queue behind
this pass's stores. Spikes are written in-place over the consumed x
tile so the whole input stays SBUF-resident (8 groups x 2 MiB = 16 MiB
+ 4 MiB work pools < 26 MiB usable).

DRAM layouts (independent knobs, host-side permute in kernel()):
  btd: [B_SH, T, D] natural layout; per-(group, batch) 1 MiB DMAs with
       2 KiB per-partition runs inside a contiguous 1 MiB window.
  gp:  [NG, P, JG] group-major; per-group single 2 MiB DMA with 16 KiB
       per-partition runs inside a contiguous 2 MiB window.
(Measured: fully partition-major [P, J] hurts loads 43.8->51.0us --
scattered 128 KiB-strided chunks lose HBM locality; stores are
insensitive, ~50us in every layout.)

Per step ([128, 1024] f32 tile, 2 batch x 512 d-elems per partition):
  DVE  scalar_tensor_tensor: acc = (mem * tau) + x_t
  ACT  Sign:  sgn = sign(acc - vth)
  ACT  Relu:  spk = relu(sgn)          (in-place over x_t)
  DVE  scalar_tensor_tensor: mem' = (acc <= vth) * acc
"""

import os
import sys

sys.path.insert(0, "/opt/trn_rl_repo")

import numpy as np

from concourse import bacc, mybir, tile
from concourse.bass_utils import run_bass_kernel_spmd

TAU = 0.2
VTH = 0.5

B, T, D = 16, 32, 65536
N_CORES = 8
B_SH = B // N_CORES          # 2 batch rows per core
P = 128                      # SBUF partitions
FB = D // P                  # 512 d-elems per partition per batch row
F = B_SH * FB                # 1024 free elems per step-tile
J = T * F                    # 32768 per-partition elements per core

GS = int(os.environ.get("LIF_GS", "4"))   # timesteps per DMA group
NG = T // GS                 # groups per pass
JG = GS * F                  # per-group free elems (4096)

IN_LAYOUT = os.environ.get("LIF_INL", "btd")    # btd | gp
OUT_LAYOUT = os.environ.get("LIF_OUTL", "btd")  # btd | gp
DUAL_LOADS = os.environ.get("LIF_DL", "0") == "1"   # odd-group loads on scalar ring
DUAL_STORES = os.environ.get("LIF_DS", "0") == "1"  # odd-group stores on gpsimd (SWDGE)
# First TRICKLE groups' stores ride the scalar ring inline with compute,
# draining during the (write-idle) load phase instead of extending the
# store phase. Measured dose-response: K=0 {98.8, 101.3, 102.5}us,
# K=2 {98.7, 105.5}us (indistinguishable from K=0 within machine noise),
# K=3 111us (mixing penalty dominates). Default 0: phase-pure, most
# validated.
TRICKLE = int(os.environ.get("LIF_TK", "0"))
# Store-split: emit each (group, batch) store as SS chunks along tl.
# SS=2 -> 512 KiB store DMAs (best measured pure-store rate: 50.0us vs
# 51.2us at 1 MiB).
SS = int(os.environ.get("LIF_SS", "1"))

_progs = {}


def _dram_decl(nc, name, layout, kind):
    f32 = mybir.dt.float32
    if layout == "btd":
        t = nc.dram_tensor(name, [B_SH, T, D], f32, kind=kind).ap()
        r = t.rearrange("b (g tl) (p f) -> g p tl b f", tl=GS, p=P)
        # per-group list of (dram AP, sbuf selector) DMA pairs
        def pairs(g, xt_v):
            return [(r[g][:, :, b], xt_v[:, :, b]) for b in range(B_SH)]
    elif layout == "gb":
        # (group, batch)-windows laid out consecutively: the per-DMA
        # pattern is identical to btd (1 MiB window, [p, tl, f]), but
        # successive DMAs sweep DRAM monotonically with zero jumps
        t = nc.dram_tensor(name, [NG * B_SH * GS, D], f32, kind=kind).ap()
        r = t.rearrange("(g b tl) (p f) -> g b p tl f", b=B_SH, tl=GS, p=P)
        def pairs(g, xt_v):
            return [(r[g][b], xt_v[:, :, b]) for b in range(B_SH)]
    else:  # gp
        t = nc.dram_tensor(name, [NG, P, JG], f32, kind=kind).ap()
        r = t.rearrange("g p j -> g p j")
        def pairs(g, xt_v):
            return [(r[g], None)]  # None -> use whole tile
    return pairs


def _build_program(hw_loop=None, mode="full"):
    f32 = mybir.dt.float32
    nc = bacc.Bacc(
        "TRN2",
        target_bir_lowering=False,
        debug=False,
        enable_asserts=False,
        num_devices=N_CORES,
    )
    in_pairs = _dram_decl(nc, "x", IN_LAYOUT, "ExternalInput")
    out_pairs = _dram_decl(nc, "out", OUT_LAYOUT, "ExternalOutput")

    with tile.TileContext(nc) as tc:
        with (
            # full-x residency: one buffer per group, reused across passes
            tc.tile_pool(name="xt", bufs=NG) as xp,
            tc.tile_pool(name="acc", bufs=3) as ap_,
            tc.tile_pool(name="sgn", bufs=3) as gp_,
            tc.tile_pool(name="mem", bufs=2) as mp,
            tc.tile_pool(name="const", bufs=1) as cp,
        ):
            nvth = cp.tile([P, 1], f32)
            nc.gpsimd.memset(nvth[:], -VTH)

            src = None
            if mode in ("store", "store2", "store3", "storesync", "dmanodep"):
                # constant source tile for dependency-free store microbench
                src = cp.tile([P, JG], f32)
                nc.gpsimd.memset(src[:], 0.125)

            def body():
                if mode == "full":
                    one_pass_phase(
                        nc, tc, in_pairs, out_pairs, xp, ap_, gp_, mp, nvth
                    )
                elif mode == "compute":
                    compute_pass(nc, xp, ap_, gp_, mp, nvth)
                else:
                    micro_pass(nc, in_pairs, out_pairs, xp, src, mode)

            if hw_loop is None:
                body()
            else:
                # benchmarking only: repeat the full pass in a HW loop so
                # per-pass device time can be fit from wall-clock deltas
                with tc.For_i(0, hw_loop, 1):
                    body()
    nc.compile()
    return nc


def _load_group(nc, eng, in_pairs, g, xt, xt_v):
    for dram, sel in in_pairs(g, xt_v):
        eng.dma_start(out=(xt[:] if sel is None else sel), in_=dram)


def _store_group(nc, eng, out_pairs, g, src_full, src_v):
    for dram, sel in out_pairs(g, src_v):
        eng.dma_start(out=dram, in_=(src_full if sel is None else sel))


def _lif_steps(nc, xt, g, ap_, gp_, mp, nvth, mem):
    """Emit the GS recurrence steps for group g; spikes in-place in xt."""
    f32 = mybir.dt.float32
    mult = mybir.AluOpType.mult
    add = mybir.AluOpType.add
    is_le = mybir.AluOpType.is_le
    Sign = mybir.ActivationFunctionType.Sign
    Relu = mybir.ActivationFunctionType.Relu
    for tl in range(GS):
        t = g * GS + tl
        xs = xt[:, tl * F : (tl + 1) * F]
        if t == 0:
            acc = xs  # mem_{-1} = 0 -> acc = x_0
        else:
            acc = ap_.tile([P, F], f32)
            # acc = (mem * tau) + x_t
            nc.vector.scalar_tensor_tensor(
                out=acc[:], in0=mem[:], scalar=TAU, in1=xs,
                op0=mult, op1=add,
            )
        sgn = gp_.tile([P, F], f32)
        # sgn = sign(acc-vth); relu(sgn) = (acc > vth) exactly
        nc.scalar.activation(sgn[:], acc[:], Sign, bias=nvth[:])
        if t < T - 1:
            mem = mp.tile([P, F], f32)
            # mem' = (acc <= vth) * acc   (hard reset)
            nc.vector.scalar_tensor_tensor(
                out=mem[:], in0=acc[:], scalar=VTH, in1=acc[:],
                op0=is_le, op1=mult,
            )
        # spike_t overwrites x_t (in-place; x_t is dead after acc/mem)
        nc.scalar.activation(xt[:, tl * F : (tl + 1) * F], sgn[:], Relu)
    return mem


def one_pass_phase(nc, tc, in_pairs, out_pairs, xp, ap_, gp_, mp, nvth):
    """Phase-pure schedule. Base: everything on the sync ring, FIFO
    gives [loads][stores]. Optional second rings need explicit gates
    (chain_iter_dep) to preserve direction purity:
      DUAL_LOADS: odd-group loads ride the scalar ring. Gates: first
        scalar load <- prev pass's last store ("sl"); first store <-
        last scalar load ("ls"). The scalar engine dispatches its load
        instructions before its compute stream; their WAR sems are met
        during the previous store phase, so ACT compute is not stalled.
      DUAL_STORES: odd-group stores ride the gpsimd SWDGE queue, all
        emitted at the end of the body (Pool is otherwise idle). Gates:
        first gpsimd store <- last load ("lg"); next pass's sync loads
        follow by ring FIFO, scalar loads by "sg".
    """
    f32 = mybir.dt.float32
    # Phase 1: issue every load
    xts = []
    first_sc_load = last_sc_load = None
    last_load = None
    for g in range(NG):
        xt = xp.tile([P, JG], f32)
        xt_v = xt[:].rearrange("p (tl b f) -> p tl b f", tl=GS, b=B_SH)
        eng = nc.scalar if (DUAL_LOADS and g % 2) else nc.sync
        ops = []
        for dram, sel in in_pairs(g, xt_v):
            op = eng.dma_start(out=(xt[:] if sel is None else sel), in_=dram)
            ops.append(op)
        if DUAL_LOADS and g % 2:
            if first_sc_load is None:
                first_sc_load = ops[0]
                tc.chain_iter_dep("sl", ops[0].ins)
                if DUAL_STORES:
                    tc.chain_iter_dep("sg", ops[0].ins)
            last_sc_load = ops[-1]
        last_load = ops[-1]
        xts.append((xt, xt_v))
    gate_load = last_sc_load if DUAL_LOADS else last_load
    if DUAL_LOADS:
        tc.chain_iter_dep("ls", gate_load.ins)
    if DUAL_STORES:
        tc.chain_iter_dep("lg", gate_load.ins)

    # Phase 2: recurrence; sync-ring stores drain after loads by FIFO
    mem = None
    first_st = last_st = None
    for g in range(NG):
        xt, xt_v = xts[g]
        mem = _lif_steps(nc, xt, g, ap_, gp_, mp, nvth, mem)
        if DUAL_STORES and g % 2:
            continue  # stored below on the gpsimd queue
        st_eng = nc.scalar if g < TRICKLE else nc.sync
        for dram, sel in out_pairs(g, xt_v):
            if SS > 1 and sel is not None:
                step = GS // SS
                chunks = [
                    (dram[:, c * step : (c + 1) * step],
                     sel[:, c * step : (c + 1) * step])
                    for c in range(SS)
                ]
            else:
                chunks = [(dram, xt[:] if sel is None else sel)]
            for dchunk, schunk in chunks:
                op = st_eng.dma_start(out=dchunk, in_=schunk)
                if g >= TRICKLE:
                    if first_st is None:
                        first_st = op
                        if DUAL_LOADS:
                            tc.chain_iter_dep("ls", op.ins)
                    last_st = op
    if DUAL_STORES:
        first_gp = None
        for g in range(1, NG, 2):
            xt, xt_v = xts[g]
            for dram, sel in out_pairs(g, xt_v):
                op = nc.gpsimd.dma_start(
                    out=dram, in_=(xt[:] if sel is None else sel)
                )
                if first_gp is None:
                    first_gp = op
                    tc.chain_iter_dep("lg", op.ins)
                last_gp = op
        if DUAL_LOADS:
            tc.chain_iter_dep("sg", last_gp.ins)
    if DUAL_LOADS:
        tc.chain_iter_dep("sl", last_st.ins)


def compute_pass(nc, xp, ap_, gp_, mp, nvth):
    """Compute-only microbench: fabricate x via memset, no DMA."""
    f32 = mybir.dt.float32
    mem = None
    for g in range(NG):
        xt = xp.tile([P, JG], f32)
        nc.gpsimd.memset(xt[:], 0.125)
        mem = _lif_steps(nc, xt, g, ap_, gp_, mp, nvth, mem)


def micro_pass(nc, in_pairs, out_pairs, xp, src, mode):
    """DMA-throughput microbenches.

    load:      input loads only (sync ring)
    store:     stores only from a constant tile (scalar ring)
    storesync: stores only, sync ring
    phasedma:  [all loads][all stores] on sync ring, stores read the
               loaded tiles -- pure-DMA floor of the phase schedule
    dmanodep:  loads on sync + stores on scalar, no cross deps
    """
    f32 = mybir.dt.float32
    if mode == "load32":
        # two back-to-back 16 MiB load sweeps, no stores: separates
        # per-phase ramp cost from direction-flip cost
        for _rep in range(2):
            for g in range(NG):
                xt = xp.tile([P, JG], f32)
                xt_v = xt[:].rearrange(
                    "p (tl b f) -> p tl b f", tl=GS, b=B_SH
                )
                _load_group(nc, nc.sync, in_pairs, g, xt, xt_v)
        return
    if mode == "phasedma":
        xts = []
        for g in range(NG):
            xt = xp.tile([P, JG], f32)
            xt_v = xt[:].rearrange(
                "p (tl b f) -> p tl b f", tl=GS, b=B_SH
            )
            _load_group(nc, nc.sync, in_pairs, g, xt, xt_v)
            xts.append((xt, xt_v))
        for g in range(NG):
            xt, xt_v = xts[g]
            _store_group(nc, nc.sync, out_pairs, g, xt[:], xt_v)
        return
    do_load = mode in ("load", "load2", "dmanodep")
    do_store = mode in ("store", "store2", "store3", "storesync", "dmanodep")
    src_v = None
    if src is not None:
        src_v = src[:].rearrange("p (tl b f) -> p tl b f", tl=GS, b=B_SH)
    for g in range(NG):
        if do_load:
            xt = xp.tile([P, JG], f32)
            xt_v = xt[:].rearrange(
                "p (tl b f) -> p tl b f", tl=GS, b=B_SH
            )
            eng = nc.scalar if (mode == "load2" and g % 2) else nc.sync
            _load_group(nc, eng, in_pairs, g, xt, xt_v)
        if do_store:
            if mode == "storesync":
                eng = nc.sync
            elif mode == "store2":
                eng = nc.sync if g % 2 else nc.scalar
            elif mode == "store3":
                eng = nc.gpsimd if g % 2 else nc.sync
            else:
                eng = nc.scalar
            _store_group(nc, eng, out_pairs, g, src[:], src_v)


def _get_program(hw_loop=None, mode="full"):
    key = (hw_loop, mode)
    if key not in _progs:
        _progs[key] = _build_program(hw_loop, mode)
    return _progs[key]


# ---- host-side layout transforms -------------------------------------

def _to_gp(xc):
    """[B_SH, T, D] -> [NG, P, JG] group-major partition-contiguous."""
    # x3[g, p, tl*F + b*FB + f] = x[b, g*GS+tl, p*FB+f]
    return np.ascontiguousarray(
        xc.reshape(B_SH, NG, GS, P, FB)
        .transpose(1, 3, 2, 0, 4)
        .reshape(NG, P, JG)
    )


def _from_gp(oc):
    """[NG, P, JG] -> [B_SH, T, D]: inverse of _to_gp."""
    return np.ascontiguousarray(
        oc.reshape(NG, P, GS, B_SH, FB)
        .transpose(3, 0, 2, 1, 4)
        .reshape(B_SH, T, D)
    )


def _to_gb(xc):
    """[B_SH, T, D] -> [NG*B_SH*GS, D]: consecutive (g, b) windows."""
    return np.ascontiguousarray(
        xc.reshape(B_SH, NG, GS, D)
        .transpose(1, 0, 2, 3)
        .reshape(NG * B_SH * GS, D)
    )


def _from_gb(oc):
    """Inverse of _to_gb."""
    return np.ascontiguousarray(
        oc.reshape(NG, B_SH, GS, D)
        .transpose(1, 0, 2, 3)
        .reshape(B_SH, T, D)
    )


def _shard_input(xc):
    if IN_LAYOUT == "btd":
        return np.ascontiguousarray(xc)
    if IN_LAYOUT == "gb":
        return _to_gb(xc)
    return _to_gp(xc)


def _gather_output(oc):
    if OUT_LAYOUT == "btd":
        return oc
    if OUT_LAYOUT == "gb":
        return _from_gb(oc)
    return _from_gp(oc)


def device_input(x):
    """Full [B, T, D] -> axis-0 shard-concatenated device input array."""
    return np.concatenate(
        [
            _shard_input(x[i * B_SH : (i + 1) * B_SH])
            for i in range(N_CORES)
        ],
        axis=0,
    )


def device_output(o):
    """Axis-0 shard-concatenated device output -> full [B, T, D]."""
    rows = o.shape[0] // N_CORES
    return np.concatenate(
        [
            _gather_output(o[i * rows : (i + 1) * rows])
            for i in range(N_CORES)
        ],
        axis=0,
    )


def _shard(x):
    return [
        {"x": _shard_input(x[i * B_SH : (i + 1) * B_SH])}
        for i in range(N_CORES)
    ]


def kernel(x):
    x = np.asarray(x, dtype=np.float32)
    assert x.shape == (B, T, D), x.shape
    nc = _get_program()
    res = run_bass_kernel_spmd(nc, _shard(x), list(range(N_CORES)))
    return np.concatenate(
        [_gather_output(res.results[i]["out"]) for i in range(N_CORES)],
        axis=0,
    )

